# revision 1
# baseline (speedup 1.0000x reference)
"""Trainium2 Bass kernel for nn_CMCI_Mamba.

Strategy: data-parallel over the 2B=8 mamba streams (1 sequence per core).
Each launch runs 2 chained mamba layers fully on-chip in d-major layout
(features on partitions, time on the free axis); the SSM recurrence uses the
DVE tensor_tensor_scan instruction (state = dA*state + dBu along free dim),
one scan per SSM state index s (16 scans of a (128, 2048) tile per layer).
B_s/C_s row-broadcasts are fp32r PE matmuls with a stride-0 (column-
broadcast) lhsT reading xact directly; s-loop elementwise muls run in fp16
(DVE 2x mode). Host does the cheap cross-stream elementwise combines
between the 2 launches.
"""
import sys
import numpy as np
from contextlib import ExitStack

for _p in ("/opt/trn_rl_repo",):
    if _p not in sys.path:
        sys.path.insert(0, _p)

import concourse.bass as bass
import concourse.bacc as bacc
import concourse.tile as tile
from concourse import mybir
from concourse import bass_utils

T, DM, DI, DS, DR, K, NL = 2048, 64, 128, 16, 4, 4, 2
B, C = 4, 2048
FP = mybir.dt.float32
FR = mybir.dt.float32r
FH = mybir.dt.float16
AX = mybir.AluOpType
AF = mybir.ActivationFunctionType

# param blob column layout (blob is (128, 512) fp32 per layer)
_B_INW = 0        # [0:64, 0:256]    in_wT
_B_CONVW = 256    # [:, 256:260]     conv_w
_B_CONVB = 260    # [:, 260]         conv_b
_B_XPW = 261      # [:, 261:297]     xp_wT
_B_DTW = 297      # [0:4, 297:425]   dt_wT
_B_DTB = 425      # [:, 425]         dt_b
_B_ANEG = 426     # [:, 426:442]     -exp(A_log)
_B_D = 442        # [:, 442]         D
_B_OUTW = 443     # [:, 443:507]     out_wT
_BLOB_W = 512


def _pack_blob(raw, l):
    blob = np.zeros((DI, _BLOB_W), np.float32)
    blob[:DM, _B_INW:_B_INW + 2 * DI] = raw["in_w"][l].T
    blob[:, _B_CONVW:_B_CONVW + K] = raw["conv_w"][l]
    blob[:, _B_CONVB] = raw["conv_b"][l]
    blob[:, _B_XPW:_B_XPW + DR + 2 * DS] = raw["xp_w"][l].T
    blob[:DR, _B_DTW:_B_DTW + DI] = raw["dt_w"][l].T
    blob[:, _B_DTB] = raw["dt_b"][l]
    blob[:, _B_ANEG:_B_ANEG + DS] = -np.exp(raw["A_log"][l])
    blob[:, _B_D] = raw["D"][l]
    blob[:, _B_OUTW:_B_OUTW + DM] = raw["out_w"][l].T
    return blob


def _build_kernel(ctx, tc, u0T, blobs, outs):
    nc = tc.nc
    NCH = 4
    CF = T // NCH  # 512 free elems per matmul (one PSUM bank)

    const = ctx.enter_context(tc.tile_pool(name="const", bufs=1))
    big = ctx.enter_context(tc.tile_pool(name="big", bufs=1))
    ub = ctx.enter_context(tc.tile_pool(name="ub", bufs=2))
    sl = ctx.enter_context(tc.tile_pool(name="sl", bufs=3))
    ps = ctx.enter_context(tc.tile_pool(name="ps", bufs=4, space="PSUM"))

    pb = []
    for l in range(NL):
        t = const.tile([DI, _BLOB_W], FP, tag=f"pb{l}", name=f"pb{l}")
        nc.sync.dma_start(t[:], blobs[l][:])
        pb.append(t)

    u_t = ub.tile([DM, T], FP, tag="u", name="u_in")
    nc.sync.dma_start(u_t[:], u0T[:])

    for l in range(NL):
        p = pb[l]
        in_wT = p[0:DM, _B_INW:_B_INW + 2 * DI]
        convw = p[:, _B_CONVW:_B_CONVW + K]
        convb = p[:, _B_CONVB:_B_CONVB + 1]
        xp_wT = p[:, _B_XPW:_B_XPW + DR + 2 * DS]
        dt_wT = p[0:DR, _B_DTW:_B_DTW + DI]
        dt_b = p[:, _B_DTB:_B_DTB + 1]
        Aneg = p[:, _B_ANEG:_B_ANEG + DS]
        Dvec = p[:, _B_D:_B_D + 1]
        out_wT = p[:, _B_OUTW:_B_OUTW + DM]

        # rounded copies for the fp32r broadcast matmuls
        xpw_r = big.tile([DI, DR + 2 * DS], FR, tag="xpw_r", name=f"xpwr{l}")
        nc.vector.tensor_copy(xpw_r[:], xp_wT)

        xpad = big.tile([DI, T + K - 1], FP, tag="xpad", name=f"xpad{l}")
        zs = big.tile([DI, T], FH, tag="zs", name=f"zs{l}")
        ztmp = big.tile([DI, T], FH, tag="ztmp", name=f"ztmp{l}")
        nc.gpsimd.memset(xpad[:, 0:K - 1], 0.0)
        for c in range(NCH):
            cs = slice(c * CF, (c + 1) * CF)
            mm = ps.tile([DI, CF], FP, tag="mm", name=f"mmx{l}_{c}")
            nc.tensor.matmul(mm[:], in_wT[:, 0:DI], u_t[:, cs],
                             start=True, stop=True)
            nc.scalar.activation(xpad[:, K - 1 + c * CF:K - 1 + (c + 1) * CF],
                                 mm[:], AF.Copy)
            mm2 = ps.tile([DI, CF], FP, tag="mm", name=f"mmz{l}_{c}")
            nc.tensor.matmul(mm2[:], in_wT[:, DI:2 * DI], u_t[:, cs],
                             start=True, stop=True)
            nc.scalar.activation(zs[:, cs], mm2[:], AF.Sigmoid)
            nc.scalar.activation(ztmp[:, cs], mm2[:], AF.Copy)
        # zs = z * sigmoid(z)  (fp16 2x, and off the critical DVE path)
        nc.vector.tensor_mul(zs[:], zs[:], ztmp[:])

        # causal depthwise conv along t (shifts are free-axis offsets);
        # conv_b folded via the two-scalar tensor_scalar form. Chunked so
        # the chain starts as soon as the first xpad chunk lands instead of
        # waiting for the full row (kills a ~16us DVE ramp per layer).
        xconv = big.tile([DI, T], FP, tag="xconv", name=f"xconv{l}")
        xact = big.tile([DI, T], FP, tag="xact", name=f"xact{l}")
        xsg = sl.tile([DI, T], FP, tag="dA", name=f"xsg{l}")
        xact_r = big.tile([DI, T], FR, tag="xact_r", name=f"xactr{l}")
        for c in range(NCH):
            cs = slice(c * CF, (c + 1) * CF)
            base = c * CF
            nc.vector.tensor_scalar(xconv[:, cs],
                                    xpad[:, K - 1 + base:K - 1 + base + CF],
                                    convw[:, K - 1:K], convb,
                                    AX.mult, AX.add)
            for k in range(K - 1):
                nc.vector.scalar_tensor_tensor(
                    xconv[:, cs], xpad[:, k + base:k + base + CF],
                    convw[:, k:k + 1], xconv[:, cs], AX.mult, AX.add)
            # xact = xconv * sigmoid(xconv)
            nc.scalar.activation(xsg[:, cs], xconv[:, cs], AF.Sigmoid)
            nc.vector.tensor_mul(xact[:, cs], xconv[:, cs], xsg[:, cs])
            nc.vector.tensor_copy(xact_r[:, cs], xact[:, cs])

        # dt rows of the x-projection (only rows 0:4 are needed in SBUF;
        # B/C rows are recomputed by the broadcast matmuls)
        dtT = big.tile([DR, T], FP, tag="dtT", name=f"dtT{l}")  # shares "dtT" tag with yf below
        for c in range(NCH):
            cs = slice(c * CF, (c + 1) * CF)
            mm = ps.tile([DI, CF], FP, tag="mm", name=f"mmp{l}_{c}")
            nc.tensor.matmul(mm[0:DR, :], xp_wT[:, 0:DR], xact[:, cs],
                             start=True, stop=True)
            nc.scalar.activation(dtT[:, cs], mm[0:DR, :], AF.Copy)

        # softplus(v) = ln(1 + exp(v)), v = dtproj + dt_b (|v| stays far
        # from fp32 exp overflow for this model's data distribution)
        delta = big.tile([DI, T], FP, tag="delta", name=f"delta{l}")
        ev = big.tile([DI, T + K - 1], FP, tag="xpad", name=f"ev{l}")
        for c in range(NCH):
            cs = slice(c * CF, (c + 1) * CF)
            mm = ps.tile([DI, CF], FP, tag="mm", name=f"mmd{l}_{c}")
            nc.tensor.matmul(mm[:], dt_wT[:], dtT[:, cs],
                             start=True, stop=True)
            nc.scalar.activation(ev[:, cs], mm[:], AF.Exp, bias=dt_b)
        dx16 = big.tile([DI, T], FH, tag="dx16", name=f"dx16_{l}")
        for c in range(NCH):
            cs = slice(c * CF, (c + 1) * CF)
            nc.scalar.activation(delta[:, cs], ev[:, cs], AF.Ln, bias=1.0)
            nc.vector.tensor_mul(dx16[:, cs], delta[:, cs], xact[:, cs])

        yacc = big.tile([DI, T], FH, tag="yacc", name=f"yacc{l}")
        for s in range(DS):
            dA = sl.tile([DI, T], FP, tag="dA", name=f"dA{l}_{s}")
            nc.scalar.activation(dA[:], delta[:], AF.Exp,
                                 scale=Aneg[:, s:s + 1])
            # B_s broadcast: out[p,t] = sum_d xp_w[4+s,d] * xact[d,t]
            # (two 1024-wide halves so bc PSUM rotates through the shared
            # 2-bank slots instead of serializing on one 4-bank tile)
            bcol = xpw_r[:, DR + s:DR + s + 1].broadcast_to((DI, DI))
            brep16 = sl.tile([DI, T], FH, tag="brep", name=f"brep{l}_{s}")
            for hf in range(2):
                hs_ = slice(hf * 1024, (hf + 1) * 1024)
                bps = ps.tile([DI, 1024], FP, tag="mm", name=f"bps{l}_{s}_{hf}")
                for c in range(2):
                    cs = slice(c * CF, (c + 1) * CF)
                    nc.tensor.matmul(bps[:, cs], bcol,
                                     xact_r[:, hf * 1024 + c * CF:
                                            hf * 1024 + (c + 1) * CF],
                                     start=True, stop=True)
                nc.scalar.activation(brep16[:, hs_], bps[:], AF.Copy)
            dBu16 = sl.tile([DI, T], FH, tag="dBu", name=f"dBu{l}_{s}")
            nc.vector.tensor_mul(dBu16[:], dx16[:], brep16[:])
            hs16 = sl.tile([DI, T], FH, tag="hs", name=f"hs{l}_{s}")
            nc.vector.tensor_tensor_scan(hs16[:], dA[:], dBu16[:], 0.0,
                                         AX.mult, AX.add)
            ccol = xpw_r[:, DR + DS + s:DR + DS + s + 1].broadcast_to((DI, DI))
            crep16 = sl.tile([DI, T], FH, tag="crep", name=f"crep{l}_{s}")
            for hf in range(2):
                hs_ = slice(hf * 1024, (hf + 1) * 1024)
                cps = ps.tile([DI, 1024], FP, tag="mm", name=f"cps{l}_{s}_{hf}")
                for c in range(2):
                    cs = slice(c * CF, (c + 1) * CF)
                    nc.tensor.matmul(cps[:, cs], ccol,
                                     xact_r[:, hf * 1024 + c * CF:
                                            hf * 1024 + (c + 1) * CF],
                                     start=True, stop=True)
                nc.scalar.activation(crep16[:, hs_], cps[:], AF.Copy)
            if s == 0:
                nc.vector.tensor_mul(yacc[:], hs16[:], crep16[:])
            else:
                hsc16 = sl.tile([DI, T], FH, tag="hsc", name=f"hsc{l}_{s}")
                nc.vector.tensor_mul(hsc16[:], hs16[:], crep16[:])
                nc.vector.tensor_add(yacc[:], yacc[:], hsc16[:])

        # y = yacc + D*x ; y *= silu(z)
        yf = big.tile([DI, T], FP, tag="dtT", name=f"yf{l}")
        nc.vector.scalar_tensor_tensor(yf[:], xact[:], Dvec, yacc[:],
                                       AX.mult, AX.add)
        nc.vector.tensor_mul(yf[:], yf[:], zs[:])

        o_t = ub.tile([DM, T], FP, tag="u", name=f"o{l}")
        for c in range(NCH):
            cs = slice(c * CF, (c + 1) * CF)
            mm = ps.tile([DI, CF], FP, tag="mm", name=f"mmo{l}_{c}")
            nc.tensor.matmul(mm[0:DM, :], out_wT[:], yf[:, cs],
                             start=True, stop=True)
            nc.scalar.activation(o_t[:, cs], mm[0:DM, :], AF.Copy)
        nc.sync.dma_start(outs[l][:], o_t[:])
        u_t = o_t


def build_program():
    nc = bacc.Bacc("TRN2", target_bir_lowering=False, debug=False)
    u0T = nc.dram_tensor("u0T", [DM, T], FP, kind="ExternalInput").ap()
    blobs = [nc.dram_tensor(f"pblob_l{l}", [DI, _BLOB_W], FP,
                            kind="ExternalInput").ap() for l in range(NL)]
    outs = [nc.dram_tensor(f"o{l + 1}T", [DM, T], FP,
                           kind="ExternalOutput").ap() for l in range(NL)]
    with tile.TileContext(nc) as tc:
        with ExitStack() as ctx:
            _build_kernel(ctx, tc, u0T, blobs, outs)
    nc.compile()
    return nc


_PROG = None


def _get_prog():
    global _PROG
    if _PROG is None:
        _PROG = build_program()
    return _PROG


def _run_launch(u_list_T, raw, trace=False, trace_kwargs=None):
    """u_list_T: list of 8 arrays (64, 2048) f32. raw: param dict (np).
    Returns (o1_list, o2_list) of (64, 2048) arrays, and the raw result."""
    nc = _get_prog()
    blobs = [_pack_blob(raw, l) for l in range(NL)]
    in_maps = []
    for b in range(8):
        in_maps.append({
            "u0T": np.ascontiguousarray(u_list_T[b], np.float32),
            "pblob_l0": blobs[0],
            "pblob_l1": blobs[1],
        })
    res = bass_utils.run_bass_kernel_spmd(
        nc, in_maps, core_ids=list(range(8)), trace=trace,
        **(trace_kwargs or {}))
    o1 = [res.results[b]["o1T"] for b in range(8)]
    o2 = [res.results[b]["o2T"] for b in range(8)]
    return o1, o2, res


def kernel(**inputs):
    inp = {k: np.asarray(v, np.float32) for k, v in inputs.items()}
    Ms = inp["Ms_feature"]
    Pan = inp["Pan_feature"]
    h = C // 2
    rawa = {n: inp["a_" + n] for n in ("in_w", "conv_w", "conv_b", "xp_w",
                                       "dt_w", "dt_b", "A_log", "D", "out_w")}
    rawb = {n: inp["b_" + n] for n in ("in_w", "conv_w", "conv_b", "xp_w",
                                       "dt_w", "dt_b", "A_log", "D", "out_w")}

    cf1 = np.concatenate([Ms[:, :h], Pan[:, h:]], axis=1)
    cf2 = np.concatenate([Pan[:, :h], Ms[:, h:]], axis=1)
    u_list = [cf1[b].T for b in range(B)] + [cf2[b].T for b in range(B)]
    o1, o2, _ = _run_launch(u_list, rawa)
    cf1_1 = np.stack([o1[b].T for b in range(B)])
    cf2_1 = np.stack([o1[B + b].T for b in range(B)])
    cf1_2 = np.stack([o2[b].T for b in range(B)])
    cf2_2 = np.stack([o2[B + b].T for b in range(B)])
    Ms1 = np.maximum((cf1_1 + cf2_1) * 0.5 + Ms, 0.0)
    Ms2 = np.maximum((cf1_2 + cf2_2) * 0.5 + Ms1, 0.0)

    cf3 = np.stack([Pan[:, ::2], Ms2[:, 1::2]], axis=2).reshape(B, C, DM)
    cf4 = np.stack([Ms2[:, ::2], Pan[:, 1::2]], axis=2).reshape(B, C, DM)
    u_list = [cf3[b].T for b in range(B)] + [cf4[b].T for b in range(B)]
    o1, o2, _ = _run_launch(u_list, rawb)
    cf3_1 = np.stack([o1[b].T for b in range(B)])
    cf4_1 = np.stack([o1[B + b].T for b in range(B)])
    cf3_2 = np.stack([o2[b].T for b in range(B)])
    cf4_2 = np.stack([o2[B + b].T for b in range(B)])
    Pan1 = np.maximum((cf3_1 + cf4_1) * 0.5 + Pan, 0.0)
    Pan2 = np.maximum((cf3_2 + cf4_2) * 0.5 + Pan1, 0.0)
    return Ms2, Pan2

